# revision 1
# baseline (speedup 1.0000x reference)
"""BankedLinear (MoE-style banked linear) Trainium2 Bass kernel.

Math: out[n] = sum_k bank_weights[n,k] * (tensor[n] @ W[sel[n,k]] + bias[sel[n,k]])
Shapes: tensor [8192,128] f32, bank_weights [8192,2] f32, bank_selections [8192,2] int,
        weights [64,128,128] f32, bias [64,128] f32 -> out [8192,128] f32.

Strategy (data parallel over tokens, weights replicated):
  - 8 cores x 1024 tokens. The host computes routing metadata only: a
    load-balanced token->core assignment, the sort of each core's 2048
    (token,k) pairs by bank id (gather/unpermute index arrays), a bank
    capacity plan shared by all cores (SPMD: one program), and the routing
    matrix pt[b,n] = sum_k bw[n,k]*[sel[n,k]==b] used for the bias term.
  - On device per core:
      1. dma_gather sorted token rows from DRAM x -> SBUF tiles [128,128]
      2. PE-transpose each tile -> Xs^T [128(in), Ctot] in SBUF
      3. per bank b: matmul(psum[:, seg] = W_b^T @ Xs^T[:, seg]) (fp32);
         weights stream in three slices over the ACT/SP/Pool DMA paths in
         bank-processing order so early banks start as soon as possible
      4. copy psum -> Y^T SBUF [128(out), Ctot]
      5. PE-transpose Y^T back to row layout, scaling rows by the sorted
         bank_weights during the PSUM->SBUF copy; quartered DMA to scratch Y
      6. two token-half pipelines: fused dma_gather of Y rows by inverse
         permutation (k=0 and k=1), out = g0 + g1 + b_tok where
         b_tok = pt^T @ bias was computed on the PE during phase 3.
"""

import numpy as np

N, K, IN, OUT, NUM_BANKS = 8192, 2, 128, 128, 64
NCORES = 8
NLOC = N // NCORES  # tokens per core
P = 128
PSUM_FREE = 512  # max fp32 matmul moving free dim / psum bank
W_SPLITS = (16, 36, 12)  # banks per DMA path: ACT, SP, Pool (in bank order)


def _routing_plan(sel_all):
    """sel_all: [N, K] int. Balances tokens across cores to minimize per-bank
    capacity (max over cores), then builds per-core routing index arrays.
    Returns (assign [NCORES, NLOC] token ids, caps, offs, Ctot, per_core)."""
    sel_all = np.asarray(sel_all).astype(np.int64)
    gcount = np.bincount(sel_all.reshape(-1), minlength=NUM_BANKS)
    ideal = (gcount + NCORES - 1) // NCORES  # per-core target per bank
    counts = np.zeros((NCORES, NUM_BANKS), dtype=np.int64)
    fill = np.zeros(NCORES, dtype=np.int64)
    assign_lists = [[] for _ in range(NCORES)]
    for n in range(N):
        b0, b1 = int(sel_all[n, 0]), int(sel_all[n, 1])
        best, best_key = -1, None
        for c in range(NCORES):
            if fill[c] >= NLOC:
                continue
            over = max(0, counts[c, b0] + 1 - ideal[b0])
            if b1 == b0:
                over += max(0, counts[c, b0] + 2 - ideal[b0])
            else:
                over += max(0, counts[c, b1] + 1 - ideal[b1])
            key = (over, counts[c, b0] + counts[c, b1], fill[c])
            if best < 0 or key < best_key:
                best, best_key = c, key
        counts[best, b0] += 1
        counts[best, b1] += 1
        fill[best] += 1
        assign_lists[best].append(n)
    assign = np.array(assign_lists, dtype=np.int64)  # [NCORES, NLOC]

    caps = counts.max(axis=0).astype(np.int64)
    pad = (-int(caps.sum())) % P
    for i in range(pad):
        caps[i % NUM_BANKS] += 1
    Ctot = int(caps.sum())
    offs = np.concatenate([[0], np.cumsum(caps)[:-1]]).astype(np.int64)

    per_core = []
    for c in range(NCORES):
        sel = sel_all[assign[c]]                 # [NLOC, K]
        gidx = np.zeros(Ctot, dtype=np.int16)    # sorted-slot -> local token row
        inv = np.zeros((NLOC, K), dtype=np.int16)  # (token,k) -> sorted slot
        fillb = offs.copy()
        for i in range(NLOC):
            for k in range(K):
                b = sel[i, k]
                slot = fillb[b]
                fillb[b] += 1
                gidx[slot] = i
                inv[i, k] = slot
        per_core.append((gidx, inv))
    return assign, caps, offs, Ctot, per_core


def _wrap_idx(flat_idx):
    """Wrap a flat int16 index list into the [128, n//16] SWDGE layout:
    index i lives at [i % 16, i // 16], replicated across the 8 Q7 groups."""
    n = flat_idx.shape[0]
    assert n % 16 == 0
    w = flat_idx.reshape(n // 16, 16).T.astype(np.int16)  # [16, n//16]
    return np.tile(w, (8, 1))  # [128, n//16]


def _build_program(caps, offs, Ctot):
    import concourse.bacc as bacc
    import concourse.tile as tile
    from concourse import mybir, library_config
    from concourse.masks import make_identity
    from concourse.tile import add_dep_helper

    f32 = mybir.dt.float32
    i16 = mybir.dt.int16

    nblk = Ctot // P
    ntok_blk = NLOC // P
    nsplit = [0] + list(np.cumsum(W_SPLITS))  # bank boundaries of the 3 slices

    nc = bacc.Bacc(None, target_bir_lowering=False, debug=False)

    x_d = nc.declare_dram_parameter("x", [NLOC, IN], f32, isOutput=False)
    w_d = nc.declare_dram_parameter("wts", [NUM_BANKS, IN, OUT], f32, isOutput=False)
    bias_d = nc.declare_dram_parameter("biasb", [NUM_BANKS, OUT], f32, isOutput=False)
    pt_d = nc.declare_dram_parameter("ptmat", [NUM_BANKS, NLOC], f32, isOutput=False)
    bws_d = nc.declare_dram_parameter("bws", [Ctot, 1], f32, isOutput=False)
    gidx_d = nc.declare_dram_parameter("gidx", [P, Ctot // 16], i16, isOutput=False)
    ginv_d = nc.declare_dram_parameter("ginv", [P, (2 * NLOC) // 16], i16,
                                       isOutput=False)
    out_d = nc.declare_dram_parameter("out", [NLOC, OUT], f32, isOutput=True)
    y_d = nc.dram_tensor("yscratch", [Ctot, OUT], f32)

    # psum column groups: per-bank column chunks (<=512 each for the psum
    # bank limit) packed into <=512-wide psum tiles
    chunks = []  # (bank, col_start, width)
    for b in range(NUM_BANKS):
        cb, ob = int(caps[b]), int(offs[b])
        while cb > 0:
            w = min(cb, PSUM_FREE)
            chunks.append((b, ob, w))
            ob += w
            cb -= w
    groups = []  # (col_start, width, [(bank, seg_off_in_group, cb)])
    cur = None
    for (b, ob, cb) in chunks:
        if cur is not None and (ob + cb - cur[0]) <= PSUM_FREE:
            cur[2].append((b, ob - cur[0], cb))
            cur[1] = ob + cb - cur[0]
        else:
            if cur is not None:
                groups.append(tuple(cur))
            cur = [ob, cb, [(b, 0, cb)]]
    groups.append(tuple(cur))

    with tile.TileContext(nc) as tc:
        with (
            tc.tile_pool(name="const", bufs=1) as cpool,
            tc.tile_pool(name="big", bufs=1) as bigpool,
            tc.tile_pool(name="psum_t", bufs=4, space="PSUM") as psum_t,
            tc.tile_pool(name="psum_y", bufs=3, space="PSUM") as psum_y,
            tc.tile_pool(name="psum_b", bufs=1, space="PSUM") as psum_b,
        ):
            ident = cpool.tile([P, P], f32)
            make_identity(nc, ident[:])
            # prime the ACT Copy LUT while DMAs run so the first real
            # activation op doesn't pay the table load mid-pipeline
            warm = cpool.tile([P, 1], f32)
            nc.vector.memset(warm[:], 0.0)
            nc.scalar.activation(warm[:], warm[:],
                                 mybir.ActivationFunctionType.Copy)

            gidx_sb = cpool.tile([P, Ctot // 16], i16)
            nc.sync.dma_start(out=gidx_sb[:], in_=gidx_d.ap())
            libload = nc.gpsimd.load_library(library_config.mlp)

            # Phase A: gather sorted token rows (split for earlier transposes)
            xg = bigpool.tile([P, nblk, IN], f32, tag="xg")
            halfblk = nblk // 2
            ga = nc.gpsimd.dma_gather(
                out_ap=xg[:, :halfblk, :], in_ap=x_d.ap(),
                idxs_ap=gidx_sb[:, :halfblk * 8],
                num_idxs=halfblk * P, num_idxs_reg=halfblk * P, elem_size=IN,
                single_packet=halfblk * P <= 1024,
            )
            gb = nc.gpsimd.dma_gather(
                out_ap=xg[:, halfblk:, :], in_ap=x_d.ap(),
                idxs_ap=gidx_sb[:, halfblk * 8:],
                num_idxs=(nblk - halfblk) * P, num_idxs_reg=(nblk - halfblk) * P,
                elem_size=IN, single_packet=(nblk - halfblk) * P <= 1024,
            )
            add_dep_helper(ga.ins, libload.ins, sync=False,
                           reason="gather needs mlp gpsimd library")
            add_dep_helper(gb.ins, libload.ins, sync=False,
                           reason="gather needs mlp gpsimd library")

            # weights in three bank slices: ACT ring, SP ring, Pool (SWDGE)
            w_parts = []
            for si, eng in zip(range(3), (nc.scalar, nc.sync, nc.gpsimd)):
                b0, b1 = nsplit[si], nsplit[si + 1]
                wp = bigpool.tile([P, (b1 - b0) * OUT], f32, tag=f"w{si}")
                wdma = eng.dma_start(
                    out=wp[:].rearrange("i (b o) -> i b o", o=OUT),
                    in_=w_d[b0:b1].rearrange("b i o -> i b o"),
                )
                if eng is nc.gpsimd:
                    add_dep_helper(wdma.ins, ga.ins, sync=False,
                                   reason="pool weight slice waits on x gathers")
                    add_dep_helper(wdma.ins, gb.ins, sync=False,
                                   reason="pool weight slice waits on x gathers")
                w_parts.append(wp)

            def w_slice(b):
                for si in range(3):
                    if nsplit[si] <= b < nsplit[si + 1]:
                        lo = (b - nsplit[si]) * OUT
                        return w_parts[si][:, lo:lo + OUT]
                raise AssertionError(b)

            # small loads on the SP ring after its weight slice
            ginv_sb = cpool.tile([P, (2 * NLOC) // 16], i16)
            nc.sync.dma_start(out=ginv_sb[:], in_=ginv_d.ap())
            bws_sb = cpool.tile([P, nblk, 1], f32)
            nc.sync.dma_start(out=bws_sb[:],
                              in_=bws_d.ap().rearrange("(t p) o -> p t o", p=P))
            bias_sb = cpool.tile([NUM_BANKS, OUT], f32)
            nc.sync.dma_start(out=bias_sb[:], in_=bias_d.ap())
            pt_sb = cpool.tile([NUM_BANKS, NLOC], f32)
            nc.sync.dma_start(out=pt_sb[:], in_=pt_d.ap())

            # Xs^T via PE transposes
            xsT = bigpool.tile([P, Ctot], f32, tag="xsT")
            for t in range(nblk):
                ptt = psum_t.tile([P, P], f32, tag="ptt")
                nc.tensor.transpose(out=ptt[:], in_=xg[:, t, :], identity=ident[:])
                if t % 2 == 0:
                    nc.vector.tensor_copy(xsT[:, t * P:(t + 1) * P], ptt[:])
                else:
                    nc.scalar.copy(xsT[:, t * P:(t + 1) * P], ptt[:])

            # bias-term matmuls (pt^T @ bias), early, parked in SBUF
            b_tok = bigpool.tile([P, ntok_blk, OUT], f32, tag="b_tok")
            for j in range(ntok_blk):
                pb = psum_b.tile([P, OUT], f32, tag="pb")
                nc.tensor.matmul(out=pb[:], lhsT=pt_sb[:, j * P:(j + 1) * P],
                                 rhs=bias_sb[:], start=True, stop=True)
                if j % 2 == 0:
                    nc.scalar.copy(b_tok[:, j, :], pb[:])
                else:
                    nc.vector.tensor_copy(b_tok[:, j, :], pb[:])

            # Phase B/C: per-bank matmuls into packed psum tiles, copy to Y^T
            ysT = bigpool.tile([P, Ctot], f32, tag="ysT")
            for gi, (col0, width, banks) in enumerate(groups):
                py = psum_y.tile([P, PSUM_FREE], f32, tag="py")
                for (b, so, cb) in banks:
                    nc.tensor.matmul(
                        out=py[:, so:so + cb],
                        lhsT=w_slice(b),
                        rhs=xsT[:, col0 + so: col0 + so + cb],
                        start=True, stop=True,
                    )
                h = width // 2
                if h > 0:
                    nc.vector.tensor_copy(ysT[:, col0:col0 + h], py[:, :h])
                    nc.scalar.copy(ysT[:, col0 + h:col0 + width], py[:, h:width])
                else:
                    nc.vector.tensor_copy(ysT[:, col0:col0 + width], py[:, :width])

            # Phase D: transpose Y^T back to row layout, scale rows by sorted
            # bank_weights during the PSUM->SBUF copy, quartered stores
            yrows = bigpool.tile([P, nblk, OUT], f32, tag="yrows")
            for t in range(nblk):
                ptt = psum_t.tile([P, P], f32, tag="ptt")
                nc.tensor.transpose(out=ptt[:], in_=ysT[:, t * P:(t + 1) * P],
                                    identity=ident[:])
                if t % 2 == 0:
                    nc.vector.tensor_scalar_mul(yrows[:, t, :], ptt[:],
                                                bws_sb[:, t, 0:1])
                else:
                    nc.scalar.activation(yrows[:, t, :], ptt[:],
                                         mybir.ActivationFunctionType.Copy,
                                         scale=bws_sb[:, t, 0:1])
            qb = [0, nblk // 4, nblk // 2, (3 * nblk) // 4, nblk]
            for qi in range(4):
                t0q, t1q = qb[qi], qb[qi + 1]
                eng = nc.sync if qi % 2 == 0 else nc.gpsimd
                eng.dma_start(
                    out=y_d[t0q * P:t1q * P].rearrange("(t p) o -> p t o", p=P),
                    in_=yrows[:, t0q:t1q, :])

            # Phase E: two token-half pipelines of gather -> adds -> store
            htok = ntok_blk // 2
            o_all = bigpool.tile([P, ntok_blk, OUT], f32, tag="o_all")
            for hi in range(2):
                g01 = bigpool.tile([P, ntok_blk, OUT], f32, tag=f"g01_{hi}")
                ge = nc.gpsimd.dma_gather(
                    out_ap=g01[:], in_ap=y_d.ap(),
                    idxs_ap=ginv_sb[:, hi * (NLOC // 16):(hi + 1) * (NLOC // 16)],
                    num_idxs=NLOC, num_idxs_reg=NLOC, elem_size=OUT,
                    single_packet=NLOC <= 1024,
                )
                add_dep_helper(ge.ins, libload.ins, sync=False,
                               reason="gather needs mlp gpsimd library")
                ja, jb = hi * htok, (hi + 1) * htok
                nc.vector.tensor_add(out=o_all[:, ja:jb, :],
                                     in0=g01[:, :htok, :], in1=g01[:, htok:, :])
                nc.vector.tensor_add(out=o_all[:, ja:jb, :],
                                     in0=o_all[:, ja:jb, :],
                                     in1=b_tok[:, ja:jb, :])
                eng = nc.sync if hi == 0 else nc.gpsimd
                eng.dma_start(
                    out=out_d[ja * P:jb * P].rearrange("(j p) o -> p j o", p=P),
                    in_=o_all[:, ja:jb, :])

    return nc


def _make_in_maps(tensor, bank_weights, bank_selections, bias, weights,
                  assign, caps, offs, Ctot, per_core):
    tensor = np.ascontiguousarray(tensor, dtype=np.float32)
    bank_weights = np.ascontiguousarray(bank_weights, dtype=np.float32)
    sel_all = np.asarray(bank_selections).astype(np.int64)
    weights = np.ascontiguousarray(weights, dtype=np.float32)
    bias_bf = np.ascontiguousarray(bias, dtype=np.float32)
    in_maps = []
    ntok_half = NLOC // 2
    for c in range(NCORES):
        gidx, inv = per_core[c]
        toks = assign[c]
        bw = bank_weights[toks]                             # [NLOC, K]
        sel = sel_all[toks]                                 # [NLOC, K]
        # sorted bank weights: bws[slot] = bw of the pair at that slot (0 pad)
        bws = np.zeros((Ctot, 1), dtype=np.float32)
        bws[inv.reshape(-1).astype(np.int64), 0] = bw.reshape(-1)
        # routing matrix pt[b, n] = sum_k bw[n,k] * [sel[n,k]==b]
        ptm = np.zeros((NUM_BANKS, NLOC), dtype=np.float32)
        rows = sel.reshape(-1)
        cols = np.repeat(np.arange(NLOC, dtype=np.int64), K)
        np.add.at(ptm, (rows, cols), bw.reshape(-1))
        # gather-back index order: token halves, each with its k=0 then k=1 ids
        ginv = np.concatenate([inv[:ntok_half, 0], inv[:ntok_half, 1],
                               inv[ntok_half:, 0], inv[ntok_half:, 1]])
        in_maps.append({
            "x": np.ascontiguousarray(tensor[toks]),
            "wts": weights,
            "biasb": bias_bf,
            "ptmat": ptm,
            "bws": bws,
            "gidx": _wrap_idx(gidx),
            "ginv": _wrap_idx(ginv),
        })
    return in_maps


def kernel(tensor, bank_weights, bank_selections, weights, bias):
    tensor = np.asarray(tensor)
    bank_weights = np.asarray(bank_weights)
    bank_selections = np.asarray(bank_selections)
    weights = np.asarray(weights)
    bias = np.asarray(bias)

    assign, caps, offs, Ctot, per_core = _routing_plan(bank_selections)
    nc = _build_program(caps, offs, Ctot)
    in_maps = _make_in_maps(tensor, bank_weights, bank_selections, bias, weights,
                            assign, caps, offs, Ctot, per_core)

    nc.finalize()
    from concourse.bass_utils import run_bass_kernel_spmd
    try:
        res = run_bass_kernel_spmd(nc, in_maps, list(range(NCORES)))
    except Exception:
        # one retry: a previous crashed session can leave the accelerator in
        # a transient bad state that clears on the next dispatch
        import time
        time.sleep(2.0)
        res = run_bass_kernel_spmd(nc, in_maps, list(range(NCORES)))
    out = np.empty((N, OUT), dtype=np.float32)
    for c in range(NCORES):
        out[assign[c]] = res.results[c]["out"]
    return out

